# revision 37
# baseline (speedup 1.0000x reference)
"""DeepFM Trainium2 kernel (8-core data-parallel).

Math: x entries are binary {0,1}, so the per-feature embedding gather is
linear in x:  emb[b,f] = T0[f] + x[b,f]*(T1[f]-T0[f]).  The model folds to
    h1    = relu(x @ W1eff + b1eff)           # K=64 matmul
    h2    = relu(h1 @ W2.T + b2)
    fm    = x@lin0 + c0 + 0.5*||x@S+sbase||^2 - 0.5*||x@Wcont+bc||^2

Device structure per 1024-row superblock (A=rows 0:512, B=rows 512:1024;
x^T is prepared host-side as bf16 [128, 4096] with A-features on
partitions 0:64 and B-features on 64:128):
  - mm1: 4 matmuls K=64 (A/B pairs run concurrently via tile_position)
    -> two PSUM regions [128, 2, 512] (h1 dims 0:128, 128:256)
  - extras: ONE K=128 block-diag matmul -> pse[0:100] = per-half
    [sum_e(16) | cont(32) | lin+-(2)]; the linear fm term rides along as
    0.5*((z+.5)^2 - (z-.5)^2) = z through the square drain
  - drains: ACT relu-drains h1 chunk1 + squares extras; DVE relu-drains
    h1 chunk2; both write fp8 in DoubleRow-interleaved layout
  - fm: ONE K=100 matmul with +-0.5 coefficients accumulating into a
    persistent PSUM bank (cols 2g, 2g+1 select the superblock's rows)
  - mm2: 2 fp8 DoubleRow matmuls (virtual K=256), W2 scaled by SW2
  - out drain split ACT/DVE at OSPLIT; host divides h2 by SW2
The loop is software-pipelined: iteration g runs mm2/out/store of g-1 so
no engine FIFO waits on a cross-engine round trip.  Full-array dummy
matmuls warm the PE clock (HAM) during the input DMA and the tail.
Outputs: hT bf16 [128, 8192] (h2^T * SW2), fmv f32 [16, 512].
Host adds c0 to fm and transposes hT back. PSUM: 2+2+1+2+1 = 8 banks.
Measured: 168.5 us (session-start baseline) -> ~43 us.
"""

import numpy as np
import ml_dtypes

import concourse.bass as bass
import concourse.tile as tile
from concourse import bacc, mybir
from concourse.bass_utils import run_bass_kernel_spmd

B = 65536
FEAT = 64
NUM_DISC = 62
D = 16
H1, H2 = 256, 128
NCORES = 8
BS = B // NCORES          # 8192 rows per core
NSUP = BS // 1024         # 8 superblocks of 1024 rows (512 "A" + 512 "B")

F32 = mybir.dt.float32
BF16 = mybir.dt.bfloat16
FP8 = mybir.dt.float8e4
AF = mybir.ActivationFunctionType
ALU = mybir.AluOpType
SW2 = 64.0                # fp8 scale on W2 (host divides hT by SW2)
OSPLIT = 256              # out-drain columns drained on ACT (rest on DVE)

TRACE = False
TRACE_KW = {}
LAST_RESULT = None

_cached_nc = None


def _precompute_weights(emb_tables, Wc, bc, Wf, bf, W1, b1, W2, b2):
    """Host-side weight folding, float64 for exactness."""
    T = np.asarray(emb_tables, np.float64)        # [62, 2, 16]
    Wc = np.asarray(Wc, np.float64)               # [32, 2]
    bc = np.asarray(bc, np.float64)               # [32]
    Wf = np.asarray(Wf, np.float64)               # [1, 64]
    bf = np.asarray(bf, np.float64)               # [1]
    W1 = np.asarray(W1, np.float64)               # [256, 1024]
    b1 = np.asarray(b1, np.float64)               # [256]
    W2 = np.asarray(W2, np.float64)               # [128, 256]
    b2 = np.asarray(b2, np.float64)               # [128]

    A = np.zeros((64, 1024))
    base = np.zeros(1024)
    for f in range(NUM_DISC):
        A[f, 16 * f:16 * f + 16] = T[f, 1] - T[f, 0]
        base[16 * f:16 * f + 16] = T[f, 0]
    A[62, 992:1024] = Wc[:, 0]
    A[63, 992:1024] = Wc[:, 1]
    base[992:1024] = bc

    W1eff = A @ W1.T                              # [64, 256]
    b1eff = base @ W1.T + b1                      # [256]
    S = A.reshape(64, 64, 16).sum(axis=1)         # [64, 16]
    sbase = base.reshape(64, 16).sum(axis=0)      # [16]
    Wcont = A[:, 992:1024]                        # [64, 32]

    q0 = (T[:, 0] ** 2).sum(axis=1)               # [62]
    q1 = (T[:, 1] ** 2).sum(axis=1)
    qlin = np.zeros(64)
    qlin[:NUM_DISC] = q1 - q0
    qconst = q0.sum()
    lin0 = Wf[0] - 0.5 * qlin                     # [64]
    c0 = bf[0] - 0.5 * qconst                     # scalar

    def dup(a):  # stack A-copy (parts 0:64) and B-copy (parts 64:128)
        return np.concatenate([a, a], axis=0)

    wmm1 = dup(W1eff).astype(ml_dtypes.bfloat16)           # [128, 256]

    # extras block: per half [S(16) | Wcont(32) | lin0 | lin0] = 50 cols
    blkA = np.concatenate(
        [S, Wcont, lin0[:, None], lin0[:, None]], axis=1)  # [64, 50]
    wext = np.zeros((128, 100))
    wext[0:64, 0:50] = blkA
    wext[64:128, 50:100] = blkA
    wext = wext.astype(ml_dtypes.bfloat16)

    # extras bias: sum_e -> sbase, cont -> bc, lin rows -> +-0.5
    bx = np.concatenate([sbase, bc, [0.5], [-0.5]])        # [50]
    bext = np.zeros((128, 1), np.float32)
    bext[0:50, 0] = bx
    bext[50:100, 0] = bx

    # fm coefficients: 0.5*sum_e^2 - 0.5*cont^2 + 0.5*((z+.5)^2-(z-.5)^2)
    cf = np.zeros(50)
    cf[0:16] = 0.5
    cf[16:48] = -0.5
    cf[48] = 0.5
    cf[49] = -0.5
    wcoef = np.zeros((128, 16 * NSUP))
    for g in range(NSUP):
        wcoef[0:50, 16 * g + 2 * g] = cf
        wcoef[50:100, 16 * g + 2 * g + 1] = cf
    wcoef = wcoef.astype(ml_dtypes.bfloat16)               # [128, 128]

    # mm2 stationary for fp8 DoubleRow: w2q[p, s, m] = SW2 * W2[m, 128s+p]
    w2q = SW2 * np.stack([W2[:, 0:128].T, W2[:, 128:256].T], axis=1)
    w2q = w2q.astype(ml_dtypes.float8_e4m3)                # [128, 2, 128]

    b1w = np.stack([b1eff[0:128], b1eff[128:256]], axis=1)

    # single bf16 weight pack: [wmm1(256) | wext(100) | wcoef(128)]
    wbf = np.concatenate(
        [wmm1.astype(np.float64), wext.astype(np.float64),
         wcoef.astype(np.float64)], axis=1)                # [128, 484]
    # single f32 bias pack: [b1(2) | bext(1) | b2*SW2(1)]
    bias = np.zeros((128, 4))
    bias[:, 0:2] = b1w
    bias[:, 2:3] = bext
    bias[:, 3] = SW2 * b2
    w = dict(wbf=wbf.astype(ml_dtypes.bfloat16), w2q=w2q,
             bias=bias.astype(np.float32))
    return w, float(c0)


def _pack_x(x):
    """x [B, 64] f32 -> per-core xtd [128, 4096] bf16 with
    xtd[c][64*h + f, 512*g + j] = x[c*8192 + 1024*g + 512*h + j, f]."""
    xc = np.asarray(x, np.float32).reshape(NCORES, NSUP, 2, 512, FEAT)
    xt = xc.transpose(0, 2, 4, 1, 3).reshape(NCORES, 128, NSUP * 512)
    return np.ascontiguousarray(xt).astype(ml_dtypes.bfloat16)


def _build_nc():
    nc = bacc.Bacc(None, target_bir_lowering=False)

    xtd_d = nc.declare_dram_parameter("xtd", [128, 512 * NSUP], BF16,
                                      isOutput=False)
    wbf_d = nc.declare_dram_parameter("wbf", [128, 484], BF16, isOutput=False)
    w2q_d = nc.declare_dram_parameter("w2q", [128, 2, 128], FP8, isOutput=False)
    bias_d = nc.declare_dram_parameter("bias", [128, 4], F32, isOutput=False)
    hT_d = nc.declare_dram_parameter("hT", [128, BS], BF16, isOutput=True)
    fmv_d = nc.declare_dram_parameter("fmv", [16, 512], F32, isOutput=True)

    with tile.TileContext(nc) as tc:
        from contextlib import ExitStack
        with ExitStack() as ctx:
            constp = ctx.enter_context(tc.tile_pool(name="const", bufs=1))
            xtp = ctx.enter_context(tc.tile_pool(name="xt", bufs=1))
            h1p = ctx.enter_context(tc.tile_pool(name="h1", bufs=4))
            stkp = ctx.enter_context(tc.tile_pool(name="stk", bufs=2))
            outp = ctx.enter_context(tc.tile_pool(name="outsb", bufs=4))
            colp = ctx.enter_context(tc.tile_pool(name="colsb", bufs=1))
            pp1 = ctx.enter_context(
                tc.tile_pool(name="ps1", bufs=1, space=bass.MemorySpace.PSUM))
            pp2 = ctx.enter_context(
                tc.tile_pool(name="ps2", bufs=1, space=bass.MemorySpace.PSUM))
            ppe = ctx.enter_context(
                tc.tile_pool(name="pse", bufs=1, space=bass.MemorySpace.PSUM))
            pph = ctx.enter_context(
                tc.tile_pool(name="psh", bufs=1, space=bass.MemorySpace.PSUM))
            ppc = ctx.enter_context(
                tc.tile_pool(name="psc", bufs=1, space=bass.MemorySpace.PSUM))

            # weights/biases: 3 DMAs on the scalar queue (idle
            # pre-compute), in consumption order: wbf feeds mm1 (first
            # real PE work), bias feeds the first drains, w2q feeds mm2
            wbf = constp.tile([128, 484], BF16)
            nc.scalar.dma_start(out=wbf[:], in_=wbf_d[:])
            biast = constp.tile([128, 4], F32)
            nc.scalar.dma_start(out=biast[:], in_=bias_d[:])
            w2q = constp.tile([128, 2, 128], FP8)
            nc.scalar.dma_start(out=w2q[:], in_=w2q_d[:])
            wmm1 = wbf[:, 0:256]
            wext = wbf[:, 256:356]
            wcoef = wbf[:, 356:484]
            b1 = biast[:, 0:2]
            bext = biast[:, 2:3]
            b2 = biast[:, 3:4]

            # fm accumulator, persistent across all superblocks (full
            # bank; rows 0:16 hold the real fm accumulation)
            pcol = ppc.tile([128, 512], F32)

            # PE warm-up: full-array (K=128, M=128) dummy matmuls on a
            # zeroed tile while the input DMAs stream in, so HAM reaches
            # 8/8 before real work starts.  They write the pcol bank,
            # which g=0's start=True then clears.
            dummy = constp.tile([128, 512], BF16)
            nc.gpsimd.memset(dummy[:], 0.0)
            for _ in range(7):
                nc.tensor.matmul(pcol[:], dummy[:, 0:128], dummy[:, :],
                                 start=True, stop=True,
                                 skip_group_check=True)

            # input x: one tile, two DMAs (first superblock alone so
            # compute starts early, remaining seven in one transfer)
            xtall = xtp.tile([128, NSUP, 512], BF16)
            nc.sync.dma_start(out=xtall[:, 0, :], in_=xtd_d[:, 0:512])
            nc.sync.dma_start(out=xtall[:, 1:NSUP, :],
                              in_=xtd_d[:, 512:512 * NSUP])
            xts = [xtall[:, g, :] for g in range(NSUP)]

            # Software-pipelined by one superblock: iteration g emits
            # mm1/extras/h1-drains/square for g, but mm2 + out-drain +
            # store for g-1.  This keeps each engine's strict-FIFO queue
            # free of cross-engine round-trip waits (e.g. DVE's
            # h1c1(g+1) no longer queues behind out(g), which would wait
            # on mm2(g) on the PE).
            h1qs = [None, None]
            phs = [None]
            def _mm2(gp):
                h1q = h1qs[gp % 2]
                ph = pph.tile([128, 1024], F32)
                phs[0] = ph
                nc.tensor.matmul(ph[:, 0:512], w2q[:], h1q[:, 0, :, :],
                                 start=True, stop=True,
                                 perf_mode=mybir.MatmulPerfMode.DoubleRow)
                nc.tensor.matmul(ph[:, 512:1024], w2q[:], h1q[:, 1, :, :],
                                 start=True, stop=True,
                                 perf_mode=mybir.MatmulPerfMode.DoubleRow)

            def _out(gp):
                ph = phs[0]
                outsb = outp.tile([128, 1024], BF16)
                nc.scalar.activation(outsb[:, 0:OSPLIT], ph[:, 0:OSPLIT],
                                     AF.Relu, bias=b2[:, 0:1])
                nc.vector.tensor_scalar(outsb[:, OSPLIT:1024],
                                        ph[:, OSPLIT:1024], b2[:, 0:1], 0.0,
                                        ALU.add, ALU.max)
                nc.sync.dma_start(out=hT_d[:, 1024 * gp:1024 * (gp + 1)],
                                  in_=outsb[:])

            for g in range(NSUP):
                xt = xts[g]

                # --- mm1: h1pre, A/B halves concurrent per chunk ---
                ps1t = pp1.tile([128, 2, 512], F32, tag='ps1t')
                nc.tensor.matmul(ps1t[:, 0, :], wmm1[0:64, 0:128],
                                 xt[0:64, :], start=True, stop=True)
                nc.tensor.matmul(ps1t[:, 1, :], wmm1[64:128, 0:128],
                                 xt[64:128, :], start=True, stop=True,
                                 tile_position=(64, 0))
                ps2t = pp2.tile([128, 2, 512], F32, tag='ps2t')
                nc.tensor.matmul(ps2t[:, 0, :], wmm1[0:64, 128:256],
                                 xt[0:64, :], start=True, stop=True)
                nc.tensor.matmul(ps2t[:, 1, :], wmm1[64:128, 128:256],
                                 xt[64:128, :], start=True, stop=True,
                                 tile_position=(64, 0))

                # --- extras: one K=128 block-diag matmul ---
                pse = ppe.tile([128, 512], F32)
                nc.tensor.matmul(pse[0:100, :], wext[:, :], xt[:, :],
                                 start=True, stop=True)

                # --- h1 relu drains (bias fused); fp8 out for DoubleRow
                # mm2.  h1q layout [p, half, ksub, col]:
                # ksub 0 = h1 dims 0:128, ksub 1 = dims 128:256 ---
                h1q = h1p.tile([128, 2, 2, 512], FP8, tag="h1")
                h1qs[g % 2] = h1q
                nc.scalar.activation(h1q[:, :, 0, :], ps1t[:], AF.Relu,
                                     bias=b1[:, 0:1])
                nc.vector.tensor_scalar(h1q[:, :, 1, :], ps2t[:],
                                        b1[:, 1:2], 0.0, ALU.add, ALU.max)

                # --- previous superblock's mm2 (PE) ---
                if g > 0:
                    _mm2(g - 1)
                if g == NSUP - 1:
                    # keep HAM warm through the drain-bound tail; ps1t's
                    # banks are dead once h1c0(g) has drained them
                    for _ in range(5):
                        nc.tensor.matmul(ps1t[:, 0, :], dummy[:, 0:128],
                                         dummy[:, :], start=True, stop=True,
                                         skip_group_check=True)

                # --- extras: (z+bias)^2; emitted before out(g-1) so the
                # ACT queue never stalls on mm2(g-1) ahead of it ---
                stk = stkp.tile([128, 512], BF16)
                nc.scalar.activation(stk[0:100, :], pse[0:100, :], AF.Square,
                                     bias=bext[0:100, 0:1])

                # --- previous superblock's out drain + store ---
                if g > 0:
                    _out(g - 1)

                # --- fm: accumulate +-0.5 coefficient reduction (the
                # last superblock's is deferred so the final mm2 isn't
                # queued behind it on the PE) ---
                if g < NSUP - 1:
                    nc.tensor.matmul(pcol[0:16, :],
                                     wcoef[0:100, 16 * g:16 * g + 16],
                                     stk[0:100, :],
                                     start=(g == 0), stop=False,
                                     skip_group_check=True)
                else:
                    stk_last = stk

            _mm2(NSUP - 1)
            _out(NSUP - 1)
            gl = NSUP - 1
            nc.tensor.matmul(pcol[0:16, :],
                             wcoef[0:100, 16 * gl:16 * gl + 16],
                             stk_last[0:100, :],
                             start=False, stop=True,
                             skip_group_check=True)

            # --- fm column drain, once ---
            colsb = colp.tile([16, 512], F32)
            nc.scalar.copy(colsb[:], pcol[0:16, :])
            nc.sync.dma_start(out=fmv_d[:], in_=colsb[:])

    nc.compile()
    return nc


def kernel(x, emb_tables, Wc, bc, Wf, bf, W1, b1, W2, b2):
    global _cached_nc, LAST_RESULT
    w, c0 = _precompute_weights(emb_tables, Wc, bc, Wf, bf, W1, b1, W2, b2)
    if _cached_nc is None:
        _cached_nc = _build_nc()
    nc = _cached_nc

    xtd = _pack_x(x)
    in_maps = []
    for i in range(NCORES):
        m = {"xtd": xtd[i]}
        m.update(w)
        in_maps.append(m)

    res = run_bass_kernel_spmd(nc, in_maps, list(range(NCORES)),
                               trace=TRACE, **TRACE_KW)
    LAST_RESULT = res
    out = np.empty((B, 129), np.float32)
    for i in range(NCORES):
        r = res.results[i]
        out[i * BS:(i + 1) * BS, 0] = (
            r["fmv"].astype(np.float32).reshape(-1) + c0)
        out[i * BS:(i + 1) * BS, 1:129] = (
            r["hT"].astype(np.float32).T * (1.0 / SW2))
    return out


# revision 39
# speedup vs baseline: 1.0499x; 1.0499x over previous
"""DeepFM Trainium2 kernel (8-core data-parallel).

Math: x entries are binary {0,1}, so the per-feature embedding gather is
linear in x:  emb[b,f] = T0[f] + x[b,f]*(T1[f]-T0[f]).  The model folds to
    h1    = relu(x @ W1eff + b1eff)           # K=64 matmul
    h2    = relu(h1 @ W2.T + b2)
    fm    = x@lin0 + c0 + 0.5*||x@S+sbase||^2 - 0.5*||x@Wcont+bc||^2

Device structure per 1024-row superblock (A=rows 0:512, B=rows 512:1024;
x^T is prepared host-side as bf16 [128, 4096] with A-features on
partitions 0:64 and B-features on 64:128):
  - mm1: 4 matmuls K=64 (A/B pairs run concurrently via tile_position)
    -> two PSUM regions [128, 2, 512] (h1 dims 0:128, 128:256)
  - extras: ONE K=128 block-diag matmul -> pse[0:100] = per-half
    [sum_e(16) | cont(32) | lin+-(2)]; the linear fm term rides along as
    0.5*((z+.5)^2 - (z-.5)^2) = z through the square drain
  - drains: ACT relu-drains h1 chunk1 + squares extras; DVE relu-drains
    h1 chunk2; both write fp8 in DoubleRow-interleaved layout
  - fm: ONE K=100 matmul with +-0.5 coefficients accumulating into a
    persistent PSUM bank (cols 2g, 2g+1 select the superblock's rows)
  - mm2: 2 fp8 DoubleRow matmuls (virtual K=256), W2 scaled by SW2
  - out drain split ACT/DVE at OSPLIT; host divides h2 by SW2
The loop is software-pipelined: iteration g runs mm2/out/store of g-1 so
no engine FIFO waits on a cross-engine round trip.  Full-array dummy
matmuls warm the PE clock (HAM) during the input DMA and the tail.
Outputs: hT bf16 [128, 8192] (h2^T * SW2), fmv f32 [16, 512].
Host adds c0 to fm and transposes hT back. PSUM: 2+2+1+2+1 = 8 banks.
Measured: 168.5 us (session-start baseline) -> ~43 us.
"""

import numpy as np
import ml_dtypes

import concourse.bass as bass
import concourse.tile as tile
from concourse import bacc, mybir
from concourse.bass_utils import run_bass_kernel_spmd

B = 65536
FEAT = 64
NUM_DISC = 62
D = 16
H1, H2 = 256, 128
NCORES = 8
BS = B // NCORES          # 8192 rows per core
NSUP = BS // 1024         # 8 superblocks of 1024 rows (512 "A" + 512 "B")

F32 = mybir.dt.float32
BF16 = mybir.dt.bfloat16
FP8 = mybir.dt.float8e4
AF = mybir.ActivationFunctionType
ALU = mybir.AluOpType
SW2 = 64.0                # fp8 scale on W2 (host divides hT by SW2)
OSPLIT = 256              # out-drain columns drained on ACT (rest on DVE)

TRACE = False
TRACE_KW = {}
LAST_RESULT = None

_cached_nc = None


def _precompute_weights(emb_tables, Wc, bc, Wf, bf, W1, b1, W2, b2):
    """Host-side weight folding, float64 for exactness."""
    T = np.asarray(emb_tables, np.float64)        # [62, 2, 16]
    Wc = np.asarray(Wc, np.float64)               # [32, 2]
    bc = np.asarray(bc, np.float64)               # [32]
    Wf = np.asarray(Wf, np.float64)               # [1, 64]
    bf = np.asarray(bf, np.float64)               # [1]
    W1 = np.asarray(W1, np.float64)               # [256, 1024]
    b1 = np.asarray(b1, np.float64)               # [256]
    W2 = np.asarray(W2, np.float64)               # [128, 256]
    b2 = np.asarray(b2, np.float64)               # [128]

    A = np.zeros((64, 1024))
    base = np.zeros(1024)
    for f in range(NUM_DISC):
        A[f, 16 * f:16 * f + 16] = T[f, 1] - T[f, 0]
        base[16 * f:16 * f + 16] = T[f, 0]
    A[62, 992:1024] = Wc[:, 0]
    A[63, 992:1024] = Wc[:, 1]
    base[992:1024] = bc

    W1eff = A @ W1.T                              # [64, 256]
    b1eff = base @ W1.T + b1                      # [256]
    S = A.reshape(64, 64, 16).sum(axis=1)         # [64, 16]
    sbase = base.reshape(64, 16).sum(axis=0)      # [16]
    Wcont = A[:, 992:1024]                        # [64, 32]

    q0 = (T[:, 0] ** 2).sum(axis=1)               # [62]
    q1 = (T[:, 1] ** 2).sum(axis=1)
    qlin = np.zeros(64)
    qlin[:NUM_DISC] = q1 - q0
    qconst = q0.sum()
    lin0 = Wf[0] - 0.5 * qlin                     # [64]
    c0 = bf[0] - 0.5 * qconst                     # scalar

    def dup(a):  # stack A-copy (parts 0:64) and B-copy (parts 64:128)
        return np.concatenate([a, a], axis=0)

    wmm1 = dup(W1eff).astype(ml_dtypes.bfloat16)           # [128, 256]

    # extras block: per half [S(16) | Wcont(32) | lin0 | lin0] = 50 cols
    blkA = np.concatenate(
        [S, Wcont, lin0[:, None], lin0[:, None]], axis=1)  # [64, 50]
    wext = np.zeros((128, 100))
    wext[0:64, 0:50] = blkA
    wext[64:128, 50:100] = blkA
    wext = wext.astype(ml_dtypes.bfloat16)

    # extras bias: sum_e -> sbase, cont -> bc, lin rows -> +-0.5
    bx = np.concatenate([sbase, bc, [0.5], [-0.5]])        # [50]
    bext = np.zeros((128, 1), np.float32)
    bext[0:50, 0] = bx
    bext[50:100, 0] = bx

    # fm coefficients: 0.5*sum_e^2 - 0.5*cont^2 + 0.5*((z+.5)^2-(z-.5)^2)
    cf = np.zeros(50)
    cf[0:16] = 0.5
    cf[16:48] = -0.5
    cf[48] = 0.5
    cf[49] = -0.5
    wcoef = np.zeros((128, 16 * NSUP))
    for g in range(NSUP):
        wcoef[0:50, 16 * g + 2 * g] = cf
        wcoef[50:100, 16 * g + 2 * g + 1] = cf
    wcoef = wcoef.astype(ml_dtypes.bfloat16)               # [128, 128]

    # mm2 stationary for fp8 DoubleRow: w2q[p, s, m] = SW2 * W2[m, 128s+p]
    w2q = SW2 * np.stack([W2[:, 0:128].T, W2[:, 128:256].T], axis=1)
    w2q = w2q.astype(ml_dtypes.float8_e4m3)                # [128, 2, 128]

    b1w = np.stack([b1eff[0:128], b1eff[128:256]], axis=1)

    # single bf16 weight pack: [wmm1(256) | wext(100) | wcoef(128)]
    wbf = np.concatenate(
        [wmm1.astype(np.float64), wext.astype(np.float64),
         wcoef.astype(np.float64)], axis=1)                # [128, 484]
    # single f32 bias pack: [b1(2) | bext(1) | b2*SW2(1)]
    bias = np.zeros((128, 4))
    bias[:, 0:2] = b1w
    bias[:, 2:3] = bext
    bias[:, 3] = SW2 * b2
    w = dict(wbf=wbf.astype(ml_dtypes.bfloat16), w2q=w2q,
             bias=bias.astype(np.float32))
    return w, float(c0)


def _pack_x(x):
    """x [B, 64] f32 -> per-core xtd [128, 4096] bf16 with
    xtd[c][64*h + f, 512*g + j] = x[c*8192 + 1024*g + 512*h + j, f]."""
    xc = np.asarray(x, np.float32).reshape(NCORES, NSUP, 2, 512, FEAT)
    xt = xc.transpose(0, 2, 4, 1, 3).reshape(NCORES, 128, NSUP * 512)
    return np.ascontiguousarray(xt).astype(ml_dtypes.bfloat16)


def _build_nc():
    nc = bacc.Bacc(None, target_bir_lowering=False)

    xtd_d = nc.declare_dram_parameter("xtd", [128, 512 * NSUP], BF16,
                                      isOutput=False)
    wbf_d = nc.declare_dram_parameter("wbf", [128, 484], BF16, isOutput=False)
    w2q_d = nc.declare_dram_parameter("w2q", [128, 2, 128], FP8, isOutput=False)
    bias_d = nc.declare_dram_parameter("bias", [128, 4], F32, isOutput=False)
    hT_d = nc.declare_dram_parameter("hT", [128, BS], FP8, isOutput=True)
    fmv_d = nc.declare_dram_parameter("fmv", [16, 512], F32, isOutput=True)

    with tile.TileContext(nc) as tc:
        from contextlib import ExitStack
        with ExitStack() as ctx:
            constp = ctx.enter_context(tc.tile_pool(name="const", bufs=1))
            xtp = ctx.enter_context(tc.tile_pool(name="xt", bufs=1))
            h1p = ctx.enter_context(tc.tile_pool(name="h1", bufs=4))
            stkp = ctx.enter_context(tc.tile_pool(name="stk", bufs=2))
            outp = ctx.enter_context(tc.tile_pool(name="outsb", bufs=4))
            colp = ctx.enter_context(tc.tile_pool(name="colsb", bufs=1))
            pp1 = ctx.enter_context(
                tc.tile_pool(name="ps1", bufs=1, space=bass.MemorySpace.PSUM))
            pp2 = ctx.enter_context(
                tc.tile_pool(name="ps2", bufs=1, space=bass.MemorySpace.PSUM))
            ppe = ctx.enter_context(
                tc.tile_pool(name="pse", bufs=1, space=bass.MemorySpace.PSUM))
            pph = ctx.enter_context(
                tc.tile_pool(name="psh", bufs=1, space=bass.MemorySpace.PSUM))
            ppc = ctx.enter_context(
                tc.tile_pool(name="psc", bufs=1, space=bass.MemorySpace.PSUM))

            # weights/biases: 3 DMAs on the scalar queue (idle
            # pre-compute), in consumption order: wbf feeds mm1 (first
            # real PE work), bias feeds the first drains, w2q feeds mm2
            wbf = constp.tile([128, 484], BF16)
            nc.scalar.dma_start(out=wbf[:], in_=wbf_d[:])
            biast = constp.tile([128, 4], F32)
            nc.scalar.dma_start(out=biast[:], in_=bias_d[:])
            w2q = constp.tile([128, 2, 128], FP8)
            nc.scalar.dma_start(out=w2q[:], in_=w2q_d[:])
            wmm1 = wbf[:, 0:256]
            wext = wbf[:, 256:356]
            wcoef = wbf[:, 356:484]
            b1 = biast[:, 0:2]
            bext = biast[:, 2:3]
            b2 = biast[:, 3:4]

            # fm accumulator, persistent across all superblocks (full
            # bank; rows 0:16 hold the real fm accumulation)
            pcol = ppc.tile([128, 512], F32)

            # PE warm-up: full-array (K=128, M=128) dummy matmuls on a
            # zeroed tile while the input DMAs stream in, so HAM reaches
            # 8/8 before real work starts.  They write the pcol bank,
            # which g=0's start=True then clears.
            dummy = constp.tile([128, 512], BF16)
            nc.gpsimd.memset(dummy[:], 0.0)
            for _ in range(7):
                nc.tensor.matmul(pcol[:], dummy[:, 0:128], dummy[:, :],
                                 start=True, stop=True,
                                 skip_group_check=True)

            # input x: one tile, two DMAs (first superblock alone so
            # compute starts early, remaining seven in one transfer)
            xtall = xtp.tile([128, NSUP, 512], BF16)
            nc.sync.dma_start(out=xtall[:, 0, :], in_=xtd_d[:, 0:512])
            nc.sync.dma_start(out=xtall[:, 1:NSUP, :],
                              in_=xtd_d[:, 512:512 * NSUP])
            xts = [xtall[:, g, :] for g in range(NSUP)]

            # Software-pipelined by one superblock: iteration g emits
            # mm1/extras/h1-drains/square for g, but mm2 + out-drain +
            # store for g-1.  This keeps each engine's strict-FIFO queue
            # free of cross-engine round-trip waits (e.g. DVE's
            # h1c1(g+1) no longer queues behind out(g), which would wait
            # on mm2(g) on the PE).
            h1qs = [None, None]
            phs = [None]
            def _mm2(gp):
                h1q = h1qs[gp % 2]
                ph = pph.tile([128, 1024], F32)
                phs[0] = ph
                nc.tensor.matmul(ph[:, 0:512], w2q[:], h1q[:, 0, :, :],
                                 start=True, stop=True,
                                 perf_mode=mybir.MatmulPerfMode.DoubleRow)
                nc.tensor.matmul(ph[:, 512:1024], w2q[:], h1q[:, 1, :, :],
                                 start=True, stop=True,
                                 perf_mode=mybir.MatmulPerfMode.DoubleRow)

            def _out(gp):
                ph = phs[0]
                outsb = outp.tile([128, 1024], FP8)
                nc.scalar.activation(outsb[:, 0:OSPLIT], ph[:, 0:OSPLIT],
                                     AF.Relu, bias=b2[:, 0:1])
                nc.vector.tensor_scalar(outsb[:, OSPLIT:1024],
                                        ph[:, OSPLIT:1024], b2[:, 0:1], 0.0,
                                        ALU.add, ALU.max)
                nc.sync.dma_start(out=hT_d[:, 1024 * gp:1024 * (gp + 1)],
                                  in_=outsb[:])

            for g in range(NSUP):
                xt = xts[g]

                # --- mm1: h1pre, A/B halves concurrent per chunk ---
                ps1t = pp1.tile([128, 2, 512], F32, tag='ps1t')
                nc.tensor.matmul(ps1t[:, 0, :], wmm1[0:64, 0:128],
                                 xt[0:64, :], start=True, stop=True)
                nc.tensor.matmul(ps1t[:, 1, :], wmm1[64:128, 0:128],
                                 xt[64:128, :], start=True, stop=True,
                                 tile_position=(64, 0))
                ps2t = pp2.tile([128, 2, 512], F32, tag='ps2t')
                nc.tensor.matmul(ps2t[:, 0, :], wmm1[0:64, 128:256],
                                 xt[0:64, :], start=True, stop=True)
                nc.tensor.matmul(ps2t[:, 1, :], wmm1[64:128, 128:256],
                                 xt[64:128, :], start=True, stop=True,
                                 tile_position=(64, 0))

                # --- extras: one K=128 block-diag matmul ---
                pse = ppe.tile([128, 512], F32)
                nc.tensor.matmul(pse[0:100, :], wext[:, :], xt[:, :],
                                 start=True, stop=True)

                # --- h1 relu drains (bias fused); fp8 out for DoubleRow
                # mm2.  h1q layout [p, half, ksub, col]:
                # ksub 0 = h1 dims 0:128, ksub 1 = dims 128:256 ---
                h1q = h1p.tile([128, 2, 2, 512], FP8, tag="h1")
                h1qs[g % 2] = h1q
                nc.scalar.activation(h1q[:, :, 0, :], ps1t[:], AF.Relu,
                                     bias=b1[:, 0:1])
                nc.vector.tensor_scalar(h1q[:, :, 1, :], ps2t[:],
                                        b1[:, 1:2], 0.0, ALU.add, ALU.max)

                # --- previous superblock's mm2 (PE) ---
                if g > 0:
                    _mm2(g - 1)
                if g == NSUP - 1:
                    # keep HAM warm through the drain-bound tail; ps1t's
                    # banks are dead once h1c0(g) has drained them
                    for _ in range(5):
                        nc.tensor.matmul(ps1t[:, 0, :], dummy[:, 0:128],
                                         dummy[:, :], start=True, stop=True,
                                         skip_group_check=True)

                # --- extras: (z+bias)^2; emitted before out(g-1) so the
                # ACT queue never stalls on mm2(g-1) ahead of it ---
                stk = stkp.tile([128, 512], BF16)
                nc.scalar.activation(stk[0:100, :], pse[0:100, :], AF.Square,
                                     bias=bext[0:100, 0:1])

                # --- previous superblock's out drain + store ---
                if g > 0:
                    _out(g - 1)

                # --- fm: accumulate +-0.5 coefficient reduction ---
                nc.tensor.matmul(pcol[0:16, :],
                                 wcoef[0:100, 16 * g:16 * g + 16],
                                 stk[0:100, :],
                                 start=(g == 0), stop=(g == NSUP - 1),
                                 skip_group_check=True)

            # --- fm column drain before the final mm2, freeing the pse
            # and pcol banks for it ---
            colsb = colp.tile([16, 512], F32)
            nc.scalar.copy(colsb[:], pcol[0:16, :])
            nc.sync.dma_start(out=fmv_d[:], in_=colsb[:])

            # --- final superblock's mm2 into the dead pse/pcol banks so
            # it need not wait for out(NSUP-2)'s drains, with an even
            # ACT/DVE split of its out drain ---
            gl = NSUP - 1
            h1q = h1qs[gl % 2]
            phA7 = ppe.tile([128, 512], F32, tag='pse')
            phB7 = ppc.tile([128, 512], F32, tag='pcol')
            nc.tensor.matmul(phA7[:], w2q[:], h1q[:, 0, :, :],
                             start=True, stop=True,
                             perf_mode=mybir.MatmulPerfMode.DoubleRow)
            nc.tensor.matmul(phB7[:], w2q[:], h1q[:, 1, :, :],
                             start=True, stop=True,
                             perf_mode=mybir.MatmulPerfMode.DoubleRow)
            outsb7 = outp.tile([128, 1024], FP8)
            nc.scalar.activation(outsb7[:, 0:512], phA7[:],
                                 AF.Relu, bias=b2[:, 0:1])
            nc.vector.tensor_scalar(outsb7[:, 512:1024], phB7[:],
                                    b2[:, 0:1], 0.0, ALU.add, ALU.max)
            nc.sync.dma_start(out=hT_d[:, 1024 * gl:1024 * (gl + 1)],
                              in_=outsb7[:])

    nc.compile()
    return nc


def kernel(x, emb_tables, Wc, bc, Wf, bf, W1, b1, W2, b2):
    global _cached_nc, LAST_RESULT
    w, c0 = _precompute_weights(emb_tables, Wc, bc, Wf, bf, W1, b1, W2, b2)
    if _cached_nc is None:
        _cached_nc = _build_nc()
    nc = _cached_nc

    xtd = _pack_x(x)
    in_maps = []
    for i in range(NCORES):
        m = {"xtd": xtd[i]}
        m.update(w)
        in_maps.append(m)

    res = run_bass_kernel_spmd(nc, in_maps, list(range(NCORES)),
                               trace=TRACE, **TRACE_KW)
    LAST_RESULT = res
    out = np.empty((B, 129), np.float32)
    for i in range(NCORES):
        r = res.results[i]
        out[i * BS:(i + 1) * BS, 0] = (
            r["fmv"].astype(np.float32).reshape(-1) + c0)
        out[i * BS:(i + 1) * BS, 1:129] = (
            r["hT"].astype(np.float32).T * (1.0 / SW2))
    return out


# revision 42
# speedup vs baseline: 1.0663x; 1.0156x over previous
"""DeepFM Trainium2 kernel (8-core data-parallel).

Math: x entries are binary {0,1}, so the per-feature embedding gather is
linear in x:  emb[b,f] = T0[f] + x[b,f]*(T1[f]-T0[f]).  The model folds to
    h1    = relu(x @ W1eff + b1eff)           # K=64 matmul
    h2    = relu(h1 @ W2.T + b2)
    fm    = x@lin0 + c0 + 0.5*||x@S+sbase||^2 - 0.5*||x@Wcont+bc||^2

Device structure per 1024-row superblock (A=rows 0:512, B=rows 512:1024;
x^T is prepared host-side as bf16 [128, 4096] with A-features on
partitions 0:64 and B-features on 64:128):
  - mm1: 4 matmuls K=64 (A/B pairs run concurrently via tile_position)
    -> two PSUM regions [128, 2, 512] (h1 dims 0:128, 128:256)
  - extras: ONE K=128 block-diag matmul -> pse[0:100] = per-half
    [sum_e(16) | cont(32) | lin+-(2)]; the linear fm term rides along as
    0.5*((z+.5)^2 - (z-.5)^2) = z through the square drain
  - drains: ACT relu-drains h1 chunk1 + squares extras; DVE relu-drains
    h1 chunk2; both write fp8 in DoubleRow-interleaved layout
  - fm: ONE K=100 matmul with +-0.5 coefficients accumulating into a
    persistent PSUM bank (cols 2g, 2g+1 select the superblock's rows)
  - mm2: 2 fp8 DoubleRow matmuls (virtual K=256), W2 scaled by SW2
  - out drain split ACT/DVE at OSPLIT; host divides h2 by SW2
The loop is software-pipelined: iteration g runs mm2/out/store of g-1 so
no engine FIFO waits on a cross-engine round trip.  Full-array dummy
matmuls warm the PE clock (HAM) during the input DMA and the tail.  The
final superblock's mm2 runs in the freed pse/pcol banks (the fm column
is drained just before), overlapping the previous superblock's drains.
Outputs: hT fp8e4 [128, 8192] (h2^T * SW2), fmv f32 [16, 512].
Host adds c0 to fm and transposes hT back. PSUM: 2+2+1+2+1 = 8 banks.
Measured: 168.5 us (session-start baseline) -> 42.2 us.
"""

import numpy as np
import ml_dtypes

import concourse.bass as bass
import concourse.tile as tile
from concourse import bacc, mybir
from concourse.bass_utils import run_bass_kernel_spmd

B = 65536
FEAT = 64
NUM_DISC = 62
D = 16
H1, H2 = 256, 128
NCORES = 8
BS = B // NCORES          # 8192 rows per core
NSUP = BS // 1024         # 8 superblocks of 1024 rows (512 "A" + 512 "B")

F32 = mybir.dt.float32
BF16 = mybir.dt.bfloat16
FP8 = mybir.dt.float8e4
AF = mybir.ActivationFunctionType
ALU = mybir.AluOpType
SW2 = 64.0                # fp8 scale on W2 (host divides hT by SW2)
OSPLIT = 256              # out-drain columns drained on ACT (rest on DVE)

TRACE = False
TRACE_KW = {}
LAST_RESULT = None

_cached_nc = None


def _precompute_weights(emb_tables, Wc, bc, Wf, bf, W1, b1, W2, b2):
    """Host-side weight folding, float64 for exactness."""
    T = np.asarray(emb_tables, np.float64)        # [62, 2, 16]
    Wc = np.asarray(Wc, np.float64)               # [32, 2]
    bc = np.asarray(bc, np.float64)               # [32]
    Wf = np.asarray(Wf, np.float64)               # [1, 64]
    bf = np.asarray(bf, np.float64)               # [1]
    W1 = np.asarray(W1, np.float64)               # [256, 1024]
    b1 = np.asarray(b1, np.float64)               # [256]
    W2 = np.asarray(W2, np.float64)               # [128, 256]
    b2 = np.asarray(b2, np.float64)               # [128]

    A = np.zeros((64, 1024))
    base = np.zeros(1024)
    for f in range(NUM_DISC):
        A[f, 16 * f:16 * f + 16] = T[f, 1] - T[f, 0]
        base[16 * f:16 * f + 16] = T[f, 0]
    A[62, 992:1024] = Wc[:, 0]
    A[63, 992:1024] = Wc[:, 1]
    base[992:1024] = bc

    W1eff = A @ W1.T                              # [64, 256]
    b1eff = base @ W1.T + b1                      # [256]
    S = A.reshape(64, 64, 16).sum(axis=1)         # [64, 16]
    sbase = base.reshape(64, 16).sum(axis=0)      # [16]
    Wcont = A[:, 992:1024]                        # [64, 32]

    q0 = (T[:, 0] ** 2).sum(axis=1)               # [62]
    q1 = (T[:, 1] ** 2).sum(axis=1)
    qlin = np.zeros(64)
    qlin[:NUM_DISC] = q1 - q0
    qconst = q0.sum()
    lin0 = Wf[0] - 0.5 * qlin                     # [64]
    c0 = bf[0] - 0.5 * qconst                     # scalar

    def dup(a):  # stack A-copy (parts 0:64) and B-copy (parts 64:128)
        return np.concatenate([a, a], axis=0)

    wmm1 = dup(W1eff).astype(ml_dtypes.bfloat16)           # [128, 256]

    # extras block: per half [S(16) | Wcont(32) | lin0 | lin0] = 50 cols
    blkA = np.concatenate(
        [S, Wcont, lin0[:, None], lin0[:, None]], axis=1)  # [64, 50]
    wext = np.zeros((128, 100))
    wext[0:64, 0:50] = blkA
    wext[64:128, 50:100] = blkA
    wext = wext.astype(ml_dtypes.bfloat16)

    # extras bias: sum_e -> sbase, cont -> bc, lin rows -> +-0.5
    bx = np.concatenate([sbase, bc, [0.5], [-0.5]])        # [50]
    bext = np.zeros((128, 1), np.float32)
    bext[0:50, 0] = bx
    bext[50:100, 0] = bx

    # fm coefficients: 0.5*sum_e^2 - 0.5*cont^2 + 0.5*((z+.5)^2-(z-.5)^2)
    cf = np.zeros(50)
    cf[0:16] = 0.5
    cf[16:48] = -0.5
    cf[48] = 0.5
    cf[49] = -0.5
    wcoef = np.zeros((128, 16 * NSUP))
    for g in range(NSUP):
        wcoef[0:50, 16 * g + 2 * g] = cf
        wcoef[50:100, 16 * g + 2 * g + 1] = cf
    wcoef = wcoef.astype(ml_dtypes.bfloat16)               # [128, 128]

    # mm2 stationary for fp8 DoubleRow: w2q[p, s, m] = SW2 * W2[m, 128s+p]
    w2q = SW2 * np.stack([W2[:, 0:128].T, W2[:, 128:256].T], axis=1)
    w2q = w2q.astype(ml_dtypes.float8_e4m3)                # [128, 2, 128]

    b1w = np.stack([b1eff[0:128], b1eff[128:256]], axis=1)

    # single bf16 weight pack: [wmm1(256) | wext(100) | wcoef(128)]
    wbf = np.concatenate(
        [wmm1.astype(np.float64), wext.astype(np.float64),
         wcoef.astype(np.float64)], axis=1)                # [128, 484]
    # single f32 bias pack: [b1(2) | bext(1) | b2*SW2(1)]
    bias = np.zeros((128, 4))
    bias[:, 0:2] = b1w
    bias[:, 2:3] = bext
    bias[:, 3] = SW2 * b2
    w = dict(wbf=wbf.astype(ml_dtypes.bfloat16), w2q=w2q,
             bias=bias.astype(np.float32))
    return w, float(c0)


def _pack_x(x):
    """x [B, 64] f32 -> per-core xtd [128, 4096] bf16 with
    xtd[c][64*h + f, 512*g + j] = x[c*8192 + 1024*g + 512*h + j, f]."""
    xc = np.asarray(x, np.float32).reshape(NCORES, NSUP, 2, 512, FEAT)
    xt = xc.transpose(0, 2, 4, 1, 3).reshape(NCORES, 128, NSUP * 512)
    return np.ascontiguousarray(xt).astype(ml_dtypes.bfloat16)


def _build_nc():
    nc = bacc.Bacc(None, target_bir_lowering=False)

    xtd_d = nc.declare_dram_parameter("xtd", [128, 512 * NSUP], BF16,
                                      isOutput=False)
    wbf_d = nc.declare_dram_parameter("wbf", [128, 484], BF16, isOutput=False)
    w2q_d = nc.declare_dram_parameter("w2q", [128, 2, 128], FP8, isOutput=False)
    bias_d = nc.declare_dram_parameter("bias", [128, 4], F32, isOutput=False)
    hT_d = nc.declare_dram_parameter("hT", [128, BS], FP8, isOutput=True)
    fmv_d = nc.declare_dram_parameter("fmv", [16, 512], F32, isOutput=True)

    with tile.TileContext(nc) as tc:
        from contextlib import ExitStack
        with ExitStack() as ctx:
            constp = ctx.enter_context(tc.tile_pool(name="const", bufs=1))
            xtp = ctx.enter_context(tc.tile_pool(name="xt", bufs=1))
            h1p = ctx.enter_context(tc.tile_pool(name="h1", bufs=4))
            stkp = ctx.enter_context(tc.tile_pool(name="stk", bufs=2))
            outp = ctx.enter_context(tc.tile_pool(name="outsb", bufs=4))
            colp = ctx.enter_context(tc.tile_pool(name="colsb", bufs=1))
            pp1 = ctx.enter_context(
                tc.tile_pool(name="ps1", bufs=1, space=bass.MemorySpace.PSUM))
            pp2 = ctx.enter_context(
                tc.tile_pool(name="ps2", bufs=1, space=bass.MemorySpace.PSUM))
            ppe = ctx.enter_context(
                tc.tile_pool(name="pse", bufs=1, space=bass.MemorySpace.PSUM))
            pph = ctx.enter_context(
                tc.tile_pool(name="psh", bufs=1, space=bass.MemorySpace.PSUM))
            ppc = ctx.enter_context(
                tc.tile_pool(name="psc", bufs=1, space=bass.MemorySpace.PSUM))

            # weights/biases: 3 DMAs on the scalar queue (idle
            # pre-compute), in consumption order: wbf feeds mm1 (first
            # real PE work), bias feeds the first drains, w2q feeds mm2
            wbf = constp.tile([128, 484], BF16)
            nc.scalar.dma_start(out=wbf[:], in_=wbf_d[:])
            biast = constp.tile([128, 4], F32)
            nc.scalar.dma_start(out=biast[:], in_=bias_d[:])
            w2q = constp.tile([128, 2, 128], FP8)
            nc.scalar.dma_start(out=w2q[:], in_=w2q_d[:])
            wmm1 = wbf[:, 0:256]
            wext = wbf[:, 256:356]
            wcoef = wbf[:, 356:484]
            b1 = biast[:, 0:2]
            bext = biast[:, 2:3]
            b2 = biast[:, 3:4]

            # fm accumulator, persistent across all superblocks (full
            # bank; rows 0:16 hold the real fm accumulation)
            pcol = ppc.tile([128, 512], F32)

            # PE warm-up: full-array (K=128, M=128) dummy matmuls on a
            # zeroed tile while the input DMAs stream in, so HAM reaches
            # 8/8 before real work starts.  They write the pcol bank,
            # which g=0's start=True then clears.
            dummy = constp.tile([128, 512], BF16)
            nc.gpsimd.memset(dummy[:], 0.0)
            for _ in range(5):
                nc.tensor.matmul(pcol[:], dummy[:, 0:128], dummy[:, :],
                                 start=True, stop=True,
                                 skip_group_check=True)

            # input x: one tile, two DMAs (first superblock alone so
            # compute starts early, remaining seven in one transfer)
            xtall = xtp.tile([128, NSUP, 512], BF16)
            nc.sync.dma_start(out=xtall[:, 0, :], in_=xtd_d[:, 0:512])
            nc.sync.dma_start(out=xtall[:, 1:NSUP, :],
                              in_=xtd_d[:, 512:512 * NSUP])
            xts = [xtall[:, g, :] for g in range(NSUP)]

            # Software-pipelined by one superblock: iteration g emits
            # mm1/extras/h1-drains/square for g, but mm2 + out-drain +
            # store for g-1.  This keeps each engine's strict-FIFO queue
            # free of cross-engine round-trip waits (e.g. DVE's
            # h1c1(g+1) no longer queues behind out(g), which would wait
            # on mm2(g) on the PE).
            h1qs = [None, None]
            phs = [None]
            def _mm2(gp):
                h1q = h1qs[gp % 2]
                ph = pph.tile([128, 1024], F32)
                phs[0] = ph
                nc.tensor.matmul(ph[:, 0:512], w2q[:], h1q[:, 0, :, :],
                                 start=True, stop=True,
                                 perf_mode=mybir.MatmulPerfMode.DoubleRow)
                nc.tensor.matmul(ph[:, 512:1024], w2q[:], h1q[:, 1, :, :],
                                 start=True, stop=True,
                                 perf_mode=mybir.MatmulPerfMode.DoubleRow)

            def _out(gp):
                ph = phs[0]
                outsb = outp.tile([128, 1024], FP8)
                nc.scalar.activation(outsb[:, 0:OSPLIT], ph[:, 0:OSPLIT],
                                     AF.Relu, bias=b2[:, 0:1])
                nc.vector.tensor_scalar(outsb[:, OSPLIT:1024],
                                        ph[:, OSPLIT:1024], b2[:, 0:1], 0.0,
                                        ALU.add, ALU.max)
                nc.sync.dma_start(out=hT_d[:, 1024 * gp:1024 * (gp + 1)],
                                  in_=outsb[:])

            for g in range(NSUP):
                xt = xts[g]

                # --- mm1: h1pre, A/B halves concurrent per chunk ---
                ps1t = pp1.tile([128, 2, 512], F32, tag='ps1t')
                nc.tensor.matmul(ps1t[:, 0, :], wmm1[0:64, 0:128],
                                 xt[0:64, :], start=True, stop=True)
                nc.tensor.matmul(ps1t[:, 1, :], wmm1[64:128, 0:128],
                                 xt[64:128, :], start=True, stop=True,
                                 tile_position=(64, 0))
                ps2t = pp2.tile([128, 2, 512], F32, tag='ps2t')
                nc.tensor.matmul(ps2t[:, 0, :], wmm1[0:64, 128:256],
                                 xt[0:64, :], start=True, stop=True)
                nc.tensor.matmul(ps2t[:, 1, :], wmm1[64:128, 128:256],
                                 xt[64:128, :], start=True, stop=True,
                                 tile_position=(64, 0))

                # --- extras: one K=128 block-diag matmul ---
                pse = ppe.tile([128, 512], F32)
                nc.tensor.matmul(pse[0:100, :], wext[:, :], xt[:, :],
                                 start=True, stop=True)

                # --- h1 relu drains (bias fused); fp8 out for DoubleRow
                # mm2.  h1q layout [p, half, ksub, col]:
                # ksub 0 = h1 dims 0:128, ksub 1 = dims 128:256 ---
                h1q = h1p.tile([128, 2, 2, 512], FP8, tag="h1")
                h1qs[g % 2] = h1q
                nc.scalar.activation(h1q[:, :, 0, :], ps1t[:], AF.Relu,
                                     bias=b1[:, 0:1])
                nc.vector.tensor_scalar(h1q[:, :, 1, :], ps2t[:],
                                        b1[:, 1:2], 0.0, ALU.add, ALU.max)

                # --- previous superblock's mm2 (PE) ---
                if g > 0:
                    _mm2(g - 1)
                if g == NSUP - 1:
                    # keep HAM warm through the drain-bound tail; ps1t's
                    # banks are dead once h1c0(g) has drained them
                    for _ in range(5):
                        nc.tensor.matmul(ps1t[:, 0, :], dummy[:, 0:128],
                                         dummy[:, :], start=True, stop=True,
                                         skip_group_check=True)

                # --- extras: (z+bias)^2; emitted before out(g-1) so the
                # ACT queue never stalls on mm2(g-1) ahead of it ---
                stk = stkp.tile([128, 512], BF16)
                nc.scalar.activation(stk[0:100, :], pse[0:100, :], AF.Square,
                                     bias=bext[0:100, 0:1])

                # --- previous superblock's out drain + store ---
                if g > 0:
                    _out(g - 1)

                # --- fm: accumulate +-0.5 coefficient reduction ---
                nc.tensor.matmul(pcol[0:16, :],
                                 wcoef[0:100, 16 * g:16 * g + 16],
                                 stk[0:100, :],
                                 start=(g == 0), stop=(g == NSUP - 1),
                                 skip_group_check=True)

            # --- fm column drain before the final mm2, freeing the pse
            # and pcol banks for it ---
            colsb = colp.tile([16, 512], F32)
            nc.scalar.copy(colsb[:], pcol[0:16, :])
            nc.sync.dma_start(out=fmv_d[:], in_=colsb[:])

            # --- final superblock's mm2 into the dead pse/pcol banks so
            # it need not wait for out(NSUP-2)'s drains, with an even
            # ACT/DVE split of its out drain ---
            gl = NSUP - 1
            h1q = h1qs[gl % 2]
            phA7 = ppe.tile([128, 512], F32, tag='pse')
            phB7 = ppc.tile([128, 512], F32, tag='pcol')
            nc.tensor.matmul(phA7[:], w2q[:], h1q[:, 0, :, :],
                             start=True, stop=True,
                             perf_mode=mybir.MatmulPerfMode.DoubleRow)
            nc.tensor.matmul(phB7[:], w2q[:], h1q[:, 1, :, :],
                             start=True, stop=True,
                             perf_mode=mybir.MatmulPerfMode.DoubleRow)
            outsb7 = outp.tile([128, 1024], FP8)
            nc.scalar.activation(outsb7[:, 0:512], phA7[:],
                                 AF.Relu, bias=b2[:, 0:1])
            # split the final store so the first half's transfer overlaps
            # the second half's drain
            nc.sync.dma_start(out=hT_d[:, 1024 * gl:1024 * gl + 512],
                              in_=outsb7[:, 0:512])
            nc.vector.tensor_scalar(outsb7[:, 512:1024], phB7[:],
                                    b2[:, 0:1], 0.0, ALU.add, ALU.max)
            nc.sync.dma_start(out=hT_d[:, 1024 * gl + 512:1024 * (gl + 1)],
                              in_=outsb7[:, 512:1024])

    nc.compile()
    return nc


def kernel(x, emb_tables, Wc, bc, Wf, bf, W1, b1, W2, b2):
    global _cached_nc, LAST_RESULT
    w, c0 = _precompute_weights(emb_tables, Wc, bc, Wf, bf, W1, b1, W2, b2)
    if _cached_nc is None:
        _cached_nc = _build_nc()
    nc = _cached_nc

    xtd = _pack_x(x)
    in_maps = []
    for i in range(NCORES):
        m = {"xtd": xtd[i]}
        m.update(w)
        in_maps.append(m)

    res = run_bass_kernel_spmd(nc, in_maps, list(range(NCORES)),
                               trace=TRACE, **TRACE_KW)
    LAST_RESULT = res
    out = np.empty((B, 129), np.float32)
    for i in range(NCORES):
        r = res.results[i]
        out[i * BS:(i + 1) * BS, 0] = (
            r["fmv"].astype(np.float32).reshape(-1) + c0)
        out[i * BS:(i + 1) * BS, 1:129] = (
            r["hT"].astype(np.float32).T * (1.0 / SW2))
    return out


# revision 43
# speedup vs baseline: 1.1159x; 1.0465x over previous
"""DeepFM Trainium2 kernel (8-core data-parallel).

Math: x entries are binary {0,1}, so the per-feature embedding gather is
linear in x:  emb[b,f] = T0[f] + x[b,f]*(T1[f]-T0[f]).  The model folds to
    h1    = relu(x @ W1eff + b1eff)           # K=64 matmul
    h2    = relu(h1 @ W2.T + b2)
    fm    = x@lin0 + c0 + 0.5*||x@S+sbase||^2 - 0.5*||x@Wcont+bc||^2

Device structure per 1024-row superblock (A=rows 0:512, B=rows 512:1024;
x^T is prepared host-side as bf16 [128, 4096] with A-features on
partitions 0:64 and B-features on 64:128):
  - mm1: 4 matmuls K=64 (A/B pairs run concurrently via tile_position)
    -> two PSUM regions [128, 2, 512] (h1 dims 0:128, 128:256)
  - extras: ONE K=128 block-diag matmul -> pse[0:100] = per-half
    [sum_e(16) | cont(32) | lin+-(2)]; the linear fm term rides along as
    0.5*((z+.5)^2 - (z-.5)^2) = z through the square drain
  - drains: ACT relu-drains h1 chunk1 + squares extras; DVE relu-drains
    h1 chunk2; both write fp8 in DoubleRow-interleaved layout
  - fm: ONE K=100 matmul with +-0.5 coefficients accumulating into a
    persistent PSUM bank (cols 2g, 2g+1 select the superblock's rows)
  - mm2: 2 fp8 DoubleRow matmuls (virtual K=256), W2 scaled by SW2
  - out drain split ACT/DVE at OSPLIT; host divides h2 by SW2
The loop is software-pipelined: iteration g runs mm2/out/store of g-1 so
no engine FIFO waits on a cross-engine round trip.  Full-array dummy
matmuls warm the PE clock (HAM) during the input DMA and the tail.  The
final superblock's mm2 runs in the freed pse/pcol banks (the fm column
is drained just before), overlapping the previous superblock's drains.
Outputs: hT fp8e4 [128, 8192] (h2^T * SW2), fmv f32 [16, 512].
Host adds c0 to fm and transposes hT back. PSUM: 2+2+1+2+1 = 8 banks.
Measured: 168.5 us (session-start baseline) -> 42.2 us.
"""

import numpy as np
import ml_dtypes

import concourse.bass as bass
import concourse.tile as tile
from concourse import bacc, mybir
from concourse.bass_utils import run_bass_kernel_spmd

B = 65536
FEAT = 64
NUM_DISC = 62
D = 16
H1, H2 = 256, 128
NCORES = 8
BS = B // NCORES          # 8192 rows per core
NSUP = BS // 1024         # 8 superblocks of 1024 rows (512 "A" + 512 "B")

F32 = mybir.dt.float32
BF16 = mybir.dt.bfloat16
FP8 = mybir.dt.float8e4
AF = mybir.ActivationFunctionType
ALU = mybir.AluOpType
SW2 = 64.0                # fp8 scale on W2 (host divides hT by SW2)
OSPLIT = 256              # out-drain columns drained on ACT (rest on DVE)

TRACE = False
TRACE_KW = {}
LAST_RESULT = None

_cached_nc = None


def _precompute_weights(emb_tables, Wc, bc, Wf, bf, W1, b1, W2, b2):
    """Host-side weight folding, float64 for exactness."""
    T = np.asarray(emb_tables, np.float64)        # [62, 2, 16]
    Wc = np.asarray(Wc, np.float64)               # [32, 2]
    bc = np.asarray(bc, np.float64)               # [32]
    Wf = np.asarray(Wf, np.float64)               # [1, 64]
    bf = np.asarray(bf, np.float64)               # [1]
    W1 = np.asarray(W1, np.float64)               # [256, 1024]
    b1 = np.asarray(b1, np.float64)               # [256]
    W2 = np.asarray(W2, np.float64)               # [128, 256]
    b2 = np.asarray(b2, np.float64)               # [128]

    A = np.zeros((64, 1024))
    base = np.zeros(1024)
    for f in range(NUM_DISC):
        A[f, 16 * f:16 * f + 16] = T[f, 1] - T[f, 0]
        base[16 * f:16 * f + 16] = T[f, 0]
    A[62, 992:1024] = Wc[:, 0]
    A[63, 992:1024] = Wc[:, 1]
    base[992:1024] = bc

    W1eff = A @ W1.T                              # [64, 256]
    b1eff = base @ W1.T + b1                      # [256]
    S = A.reshape(64, 64, 16).sum(axis=1)         # [64, 16]
    sbase = base.reshape(64, 16).sum(axis=0)      # [16]
    Wcont = A[:, 992:1024]                        # [64, 32]

    q0 = (T[:, 0] ** 2).sum(axis=1)               # [62]
    q1 = (T[:, 1] ** 2).sum(axis=1)
    qlin = np.zeros(64)
    qlin[:NUM_DISC] = q1 - q0
    qconst = q0.sum()
    lin0 = Wf[0] - 0.5 * qlin                     # [64]
    c0 = bf[0] - 0.5 * qconst                     # scalar

    def dup(a):  # stack A-copy (parts 0:64) and B-copy (parts 64:128)
        return np.concatenate([a, a], axis=0)

    wmm1 = dup(W1eff).astype(ml_dtypes.bfloat16)           # [128, 256]

    # extras block: per half [S(16) | Wcont(32) | lin0 | lin0] = 50 cols
    blkA = np.concatenate(
        [S, Wcont, lin0[:, None], lin0[:, None]], axis=1)  # [64, 50]
    wext = np.zeros((128, 100))
    wext[0:64, 0:50] = blkA
    wext[64:128, 50:100] = blkA
    wext = wext.astype(ml_dtypes.bfloat16)

    # extras bias: sum_e -> sbase, cont -> bc, lin rows -> +-0.5
    bx = np.concatenate([sbase, bc, [0.5], [-0.5]])        # [50]
    bext = np.zeros((128, 1), np.float32)
    bext[0:50, 0] = bx
    bext[50:100, 0] = bx

    # fm coefficients: 0.5*sum_e^2 - 0.5*cont^2 + 0.5*((z+.5)^2-(z-.5)^2)
    cf = np.zeros(50)
    cf[0:16] = 0.5
    cf[16:48] = -0.5
    cf[48] = 0.5
    cf[49] = -0.5
    wcoef = np.zeros((128, 16 * NSUP))
    for g in range(NSUP):
        wcoef[0:50, 16 * g + 2 * g] = cf
        wcoef[50:100, 16 * g + 2 * g + 1] = cf
    wcoef = wcoef.astype(ml_dtypes.bfloat16)               # [128, 128]

    # mm2 stationary for fp8 DoubleRow: w2q[p, s, m] = SW2 * W2[m, 128s+p]
    w2q = SW2 * np.stack([W2[:, 0:128].T, W2[:, 128:256].T], axis=1)
    w2q = w2q.astype(ml_dtypes.float8_e4m3)                # [128, 2, 128]

    b1w = np.stack([b1eff[0:128], b1eff[128:256]], axis=1)

    # single bf16 weight pack: [wmm1(256) | wext(100) | wcoef(128)]
    wbf = np.concatenate(
        [wmm1.astype(np.float64), wext.astype(np.float64),
         wcoef.astype(np.float64)], axis=1)                # [128, 484]
    # single f32 bias pack: [b1(2) | bext(1) | b2*SW2(1)]
    bias = np.zeros((128, 4))
    bias[:, 0:2] = b1w
    bias[:, 2:3] = bext
    bias[:, 3] = SW2 * b2
    w = dict(wbf=wbf.astype(ml_dtypes.bfloat16), w2q=w2q,
             bias=bias.astype(np.float32))
    return w, float(c0)


def _pack_x(x):
    """x [B, 64] f32 -> per-core xtd [128, 4096] bf16 with
    xtd[c][64*h + f, 512*g + j] = x[c*8192 + 1024*g + 512*h + j, f]."""
    xc = np.asarray(x, np.float32).reshape(NCORES, NSUP, 2, 512, FEAT)
    xt = xc.transpose(0, 2, 4, 1, 3).reshape(NCORES, 128, NSUP * 512)
    return np.ascontiguousarray(xt).astype(ml_dtypes.bfloat16)


def _build_nc():
    nc = bacc.Bacc(None, target_bir_lowering=False)

    xtd_d = nc.declare_dram_parameter("xtd", [128, 512 * NSUP], BF16,
                                      isOutput=False)
    wbf_d = nc.declare_dram_parameter("wbf", [128, 484], BF16, isOutput=False)
    w2q_d = nc.declare_dram_parameter("w2q", [128, 2, 128], FP8, isOutput=False)
    bias_d = nc.declare_dram_parameter("bias", [128, 4], F32, isOutput=False)
    hT_d = nc.declare_dram_parameter("hT", [128, BS], FP8, isOutput=True)
    fmv_d = nc.declare_dram_parameter("fmv", [16, 512], F32, isOutput=True)

    with tile.TileContext(nc) as tc:
        from contextlib import ExitStack
        with ExitStack() as ctx:
            constp = ctx.enter_context(tc.tile_pool(name="const", bufs=1))
            xtp = ctx.enter_context(tc.tile_pool(name="xt", bufs=1))
            h1p = ctx.enter_context(tc.tile_pool(name="h1", bufs=4))
            stkp = ctx.enter_context(tc.tile_pool(name="stk", bufs=2))
            outp = ctx.enter_context(tc.tile_pool(name="outsb", bufs=4))
            colp = ctx.enter_context(tc.tile_pool(name="colsb", bufs=1))
            pp1 = ctx.enter_context(
                tc.tile_pool(name="ps1", bufs=1, space=bass.MemorySpace.PSUM))
            pp2 = ctx.enter_context(
                tc.tile_pool(name="ps2", bufs=1, space=bass.MemorySpace.PSUM))
            ppe = ctx.enter_context(
                tc.tile_pool(name="pse", bufs=1, space=bass.MemorySpace.PSUM))
            pph = ctx.enter_context(
                tc.tile_pool(name="psh", bufs=1, space=bass.MemorySpace.PSUM))
            ppc = ctx.enter_context(
                tc.tile_pool(name="psc", bufs=1, space=bass.MemorySpace.PSUM))

            # weights/biases: 3 DMAs on the scalar queue (idle
            # pre-compute), in consumption order: wbf feeds mm1 (first
            # real PE work), bias feeds the first drains, w2q feeds mm2
            wbf = constp.tile([128, 484], BF16)
            nc.scalar.dma_start(out=wbf[:], in_=wbf_d[:])
            biast = constp.tile([128, 4], F32)
            nc.scalar.dma_start(out=biast[:], in_=bias_d[:])
            w2q = constp.tile([128, 2, 128], FP8)
            nc.scalar.dma_start(out=w2q[:], in_=w2q_d[:])
            wmm1 = wbf[:, 0:256]
            wext = wbf[:, 256:356]
            wcoef = wbf[:, 356:484]
            b1 = biast[:, 0:2]
            bext = biast[:, 2:3]
            b2 = biast[:, 3:4]

            # fm accumulator, persistent across all superblocks (full
            # bank; rows 0:16 hold the real fm accumulation)
            pcol = ppc.tile([128, 512], F32)

            # PE warm-up: full-array (K=128, M=128) dummy matmuls on a
            # zeroed tile while the input DMAs stream in, so HAM reaches
            # 8/8 before real work starts.  They write the pcol bank,
            # which g=0's start=True then clears.
            dummy = constp.tile([128, 512], BF16)
            nc.gpsimd.memset(dummy[:], 0.0)
            for _ in range(5):
                nc.tensor.matmul(pcol[:], dummy[:, 0:128], dummy[:, :],
                                 start=True, stop=True,
                                 skip_group_check=True)

            # input x: one tile, two DMAs (first superblock alone so
            # compute starts early, remaining seven in one transfer)
            xtall = xtp.tile([128, NSUP, 512], BF16)
            nc.sync.dma_start(out=xtall[:, 0, :], in_=xtd_d[:, 0:512])
            nc.sync.dma_start(out=xtall[:, 1:NSUP, :],
                              in_=xtd_d[:, 512:512 * NSUP])
            xts = [xtall[:, g, :] for g in range(NSUP)]

            # Software-pipelined by one superblock: iteration g emits
            # mm1/extras/h1-drains/square for g, but mm2 + out-drain +
            # store for g-1.  This keeps each engine's strict-FIFO queue
            # free of cross-engine round-trip waits (e.g. DVE's
            # h1c1(g+1) no longer queues behind out(g), which would wait
            # on mm2(g) on the PE).
            h1qs = [None, None]
            phs = [None]
            def _mm2(gp):
                h1q = h1qs[gp % 2]
                ph = pph.tile([128, 1024], F32)
                phs[0] = ph
                nc.tensor.matmul(ph[:, 0:512], w2q[:], h1q[:, 0, :, :],
                                 start=True, stop=True,
                                 perf_mode=mybir.MatmulPerfMode.DoubleRow)
                nc.tensor.matmul(ph[:, 512:1024], w2q[:], h1q[:, 1, :, :],
                                 start=True, stop=True,
                                 perf_mode=mybir.MatmulPerfMode.DoubleRow)

            def _out(gp):
                ph = phs[0]
                outsb = outp.tile([128, 1024], FP8)
                nc.scalar.activation(outsb[:, 0:OSPLIT], ph[:, 0:OSPLIT],
                                     AF.Relu, bias=b2[:, 0:1])
                nc.vector.tensor_scalar(outsb[:, OSPLIT:1024],
                                        ph[:, OSPLIT:1024], b2[:, 0:1], 0.0,
                                        ALU.add, ALU.max)
                nc.sync.dma_start(out=hT_d[:, 1024 * gp:1024 * (gp + 1)],
                                  in_=outsb[:])

            for g in range(NSUP):
                xt = xts[g]

                # --- mm1: h1pre, A/B halves concurrent per chunk ---
                ps1t = pp1.tile([128, 2, 512], F32, tag='ps1t')
                nc.tensor.matmul(ps1t[:, 0, :], wmm1[0:64, 0:128],
                                 xt[0:64, :], start=True, stop=True)
                nc.tensor.matmul(ps1t[:, 1, :], wmm1[64:128, 0:128],
                                 xt[64:128, :], start=True, stop=True,
                                 tile_position=(64, 0))
                ps2t = pp2.tile([128, 2, 512], F32, tag='ps2t')
                nc.tensor.matmul(ps2t[:, 0, :], wmm1[0:64, 128:256],
                                 xt[0:64, :], start=True, stop=True)
                nc.tensor.matmul(ps2t[:, 1, :], wmm1[64:128, 128:256],
                                 xt[64:128, :], start=True, stop=True,
                                 tile_position=(64, 0))

                # --- extras: one K=128 block-diag matmul ---
                pse = ppe.tile([128, 512], F32)
                nc.tensor.matmul(pse[0:100, :], wext[:, :], xt[:, :],
                                 start=True, stop=True)

                # --- h1 relu drains (bias fused); fp8 out for DoubleRow
                # mm2.  h1q layout [p, half, ksub, col]:
                # ksub 0 = h1 dims 0:128, ksub 1 = dims 128:256 ---
                h1q = h1p.tile([128, 2, 2, 512], FP8, tag="h1")
                h1qs[g % 2] = h1q
                nc.scalar.activation(h1q[:, :, 0, :], ps1t[:], AF.Relu,
                                     bias=b1[:, 0:1])
                nc.vector.tensor_scalar(h1q[:, :, 1, :], ps2t[:],
                                        b1[:, 1:2], 0.0, ALU.add, ALU.max)

                # --- previous superblock's mm2 (PE) ---
                if g > 0:
                    _mm2(g - 1)
                if g == NSUP - 1:
                    # keep HAM warm through the drain-bound tail; ps1t's
                    # banks are dead once h1c0(g) has drained them
                    for _ in range(5):
                        nc.tensor.matmul(ps1t[:, 0, :], dummy[:, 0:128],
                                         dummy[:, :], start=True, stop=True,
                                         skip_group_check=True)

                # --- extras: (z+bias)^2; emitted before out(g-1) so the
                # ACT queue never stalls on mm2(g-1) ahead of it ---
                stk = stkp.tile([128, 512], BF16)
                nc.scalar.activation(stk[0:100, :], pse[0:100, :], AF.Square,
                                     bias=bext[0:100, 0:1])

                # --- previous superblock's out drain + store ---
                if g > 0:
                    _out(g - 1)

                # --- fm: accumulate +-0.5 coefficient reduction ---
                nc.tensor.matmul(pcol[0:16, :],
                                 wcoef[0:100, 16 * g:16 * g + 16],
                                 stk[0:100, :],
                                 start=(g == 0), stop=(g == NSUP - 1),
                                 skip_group_check=True)

                if g == 0:
                    # bridge the pipeline-fill PE gap (mm1(1) waits on
                    # superblock 0's drains) so HAM warms up instead of
                    # cooling; the ph bank is still unused at this point
                    scrd = pph.tile([128, 1024], F32, tag='ph')
                    for _ in range(4):
                        nc.tensor.matmul(scrd[:, 0:512], dummy[:, 0:128],
                                         dummy[:, :], start=True, stop=True,
                                         skip_group_check=True)

            # --- fm column drain before the final mm2, freeing the pse
            # and pcol banks for it ---
            colsb = colp.tile([16, 512], F32)
            nc.scalar.copy(colsb[:], pcol[0:16, :])
            nc.sync.dma_start(out=fmv_d[:], in_=colsb[:])

            # --- final superblock's mm2 into the dead pse/pcol banks so
            # it need not wait for out(NSUP-2)'s drains, with an even
            # ACT/DVE split of its out drain ---
            gl = NSUP - 1
            h1q = h1qs[gl % 2]
            phA7 = ppe.tile([128, 512], F32, tag='pse')
            phB7 = ppc.tile([128, 512], F32, tag='pcol')
            nc.tensor.matmul(phA7[:], w2q[:], h1q[:, 0, :, :],
                             start=True, stop=True,
                             perf_mode=mybir.MatmulPerfMode.DoubleRow)
            nc.tensor.matmul(phB7[:], w2q[:], h1q[:, 1, :, :],
                             start=True, stop=True,
                             perf_mode=mybir.MatmulPerfMode.DoubleRow)
            outsb7 = outp.tile([128, 1024], FP8)
            nc.scalar.activation(outsb7[:, 0:512], phA7[:],
                                 AF.Relu, bias=b2[:, 0:1])
            # split the final store so the first half's transfer overlaps
            # the second half's drain
            nc.sync.dma_start(out=hT_d[:, 1024 * gl:1024 * gl + 512],
                              in_=outsb7[:, 0:512])
            nc.vector.tensor_scalar(outsb7[:, 512:1024], phB7[:],
                                    b2[:, 0:1], 0.0, ALU.add, ALU.max)
            nc.sync.dma_start(out=hT_d[:, 1024 * gl + 512:1024 * (gl + 1)],
                              in_=outsb7[:, 512:1024])

    nc.compile()
    return nc


def kernel(x, emb_tables, Wc, bc, Wf, bf, W1, b1, W2, b2):
    global _cached_nc, LAST_RESULT
    w, c0 = _precompute_weights(emb_tables, Wc, bc, Wf, bf, W1, b1, W2, b2)
    if _cached_nc is None:
        _cached_nc = _build_nc()
    nc = _cached_nc

    xtd = _pack_x(x)
    in_maps = []
    for i in range(NCORES):
        m = {"xtd": xtd[i]}
        m.update(w)
        in_maps.append(m)

    res = run_bass_kernel_spmd(nc, in_maps, list(range(NCORES)),
                               trace=TRACE, **TRACE_KW)
    LAST_RESULT = res
    out = np.empty((B, 129), np.float32)
    for i in range(NCORES):
        r = res.results[i]
        out[i * BS:(i + 1) * BS, 0] = (
            r["fmv"].astype(np.float32).reshape(-1) + c0)
        out[i * BS:(i + 1) * BS, 1:129] = (
            r["hT"].astype(np.float32).T * (1.0 / SW2))
    return out
